# revision 1
# baseline (speedup 1.0000x reference)
"""AFNO (Adaptive Fourier Neural Operator) Trainium2 kernel, v2.

Data-parallel over batch: 32 batches -> 8 cores x 4 batches.
Per core: 4 batches x 2 cq-pair groups. Each group processes TWO c-quarters
(blocks) at once:
  - W-FFT / iW-FFT use block-diagonal stacked stationaries (one matmul pass
    covers both quarters; contraction 112 of 128 rows).
  - H-FFT pairs the two quarters in one PSUM bank (rows 0:64 / 64:128 via
    tile_position) with zero-padded stationaries so the whole bank is
    written -> single [128,n] eviction.
  - The c<->spatial corner turns are PE transposes that process both
    quarters per instruction (dual-identity moving operand), batched into
    PSUM banks and evicted in ~1KB strips.
  - Block mixing is packed-complex: rows/cols interleave (2c+r) so one
    128-contraction does the full complex product.

Pipeline per group (a/b = the two c-quarters):
  x (w|h,c (2 halves))  --load 4 h-chunks (cast f32->bf16)
  S2  Wfft   (block-diag)      -> YW [116 | h,c]      (per h-chunk)
  rot1 (DMA via DRAM s1)       -> YH_a/b [112=(r,h) | w',c]
  S4  Hfft   (paired banks)    -> XF [128=(a,0,b,0) | w',c,r]
  S5  corner turn (dual)       -> XM[kc] [128=(2c+r) | H,w',h]
  M1  relu(.+b1)               -> HM
  M2  softshrink(.+b2)         -> OM
  S8  corner turn back         -> OC [112=(a h'|b h') | w',c,r]
  S9  iH (half-zero stat.)     -> VH_a/b [112=(r,h) | w',c]
  rot4 (DMA via DRAM s4)       -> VW [116 | h,c]      (per h-chunk)
  S11 iW (block-diag)          -> OUT chunks (f32) -> DRAM

HW-validated choices (A/B on trn2): no Q7/gpsimd elementwise (far slower
than modeled), few big rot1-read DMAs instead of chunked ones, f32 SBUF
staging + plain sync store instead of casting gpsimd store. PSUM->SBUF
evictions are greedily balanced between DVE and ACT by estimated cost.
"""
import numpy as np
import ml_dtypes
from contextlib import ExitStack

import concourse.bass as bass
import concourse.mybir as mybir
import concourse.tile as tile
from concourse import bacc
from concourse.bass_utils import run_bass_kernel_spmd
from concourse.masks import make_identity

H = 56
W = 56
WF = 29
C = 768
NB = 4
BS = 192
LAM = 0.01
NCORES = 8
B_FULL = 32
BPC = B_FULL // NCORES  # 4
NCH = 4                 # h-chunks per group
HCH = H // NCH          # 14
NHC = H * BS            # 10752 free elems of (h, c) per half-pair layout
NWC = WF * BS           # 5568
SPA = H * WF            # 1624 spatial per half

F32 = mybir.dt.float32
BF16 = mybir.dt.bfloat16
AF = mybir.ActivationFunctionType
ALU = mybir.AluOpType

BF = ml_dtypes.bfloat16
import os
# HW A/B tested: the Q7 (Pool) elementwise path is far slower than modeled,
# and fewer/bigger rot1-read DMAs beat chunked ones.
USE_POOL_TT = os.environ.get('KV2_POOL_TT', '0') == '1'
CHUNKED_ROT1 = os.environ.get('KV2_CHUNK_ROT1', '0') == '1'
SYNC_STORE = os.environ.get('KV2_SYNC_STORE', '1') == '1'
KC_OUTER = os.environ.get('KV2_KC_OUTER', '0') == '1'
BIG_ROT4 = os.environ.get('KV2_BIG_ROT4', '0') == '1'
BIG_LOAD = os.environ.get('KV2_BIG_LOAD', '0') == '1'
PS_ALT = os.environ.get('KV2_PS_ALT', '1') == '1'
NCHI = 2 if BIG_LOAD else NCH
HCHI = H // NCHI


def make_consts(w1, b1, w2, b2):
    """Pack DFT matrices and mixing weights/biases host-side (numpy)."""
    w = np.arange(W)
    wp = np.arange(WF)
    ang = 2 * np.pi * np.outer(wp, w) / W
    Cw = np.cos(ang) / np.sqrt(W)
    Sw = np.sin(ang) / np.sqrt(W)
    h = np.arange(H)
    angh = 2 * np.pi * np.outer(h, h) / H
    Ch = np.cos(angh) / np.sqrt(H)
    Sh = np.sin(angh) / np.sqrt(H)
    Chi, Shi = Ch, Sh
    alpha = np.full(WF, 2.0)
    alpha[0] = 1.0
    alpha[WF - 1] = 1.0
    A = (alpha[None, :] * np.cos(2 * np.pi * np.outer(w, wp) / W)) / np.sqrt(W)
    Bm = (-alpha[None, :] * np.sin(2 * np.pi * np.outer(w, wp) / W)) / np.sqrt(W)
    Bm[:, 0] = 0.0
    Bm[:, WF - 1] = 0.0

    # W-fft stationary, block-diag for the two stacked c-quarters:
    # rows 0:56 (w of quarter a) -> cols 0:58 (Yr|Yi of a); rows 56:112 -> 58:116
    fwA = np.zeros((W, 58), np.float32)
    fwA[:, :WF] = Cw.T
    fwA[:, WF:] = -Sw.T
    fw2 = np.zeros((112, 116), np.float32)
    fw2[0:56, 0:58] = fwA
    fw2[56:112, 58:116] = fwA

    # H-fft stationaries, moving rows = [Yr(h) ; Yi(h)] (112), 8 zero pad
    # cols so the matmul writes psum rows 0:64 (or 64:128) entirely.
    fhr = np.zeros((112, 64), np.float32)  # -> Xr = Ch Yr + Sh Yi
    fhr[:H, :H] = Ch.T
    fhr[H:, :H] = Sh.T
    fhi = np.zeros((112, 64), np.float32)  # -> Xi = Ch Yi - Sh Yr
    fhi[:H, :H] = -Sh.T
    fhi[H:, :H] = Ch.T

    # iH stationaries: contraction rows = oc rows (a h' 0:56 | b h' 56:112);
    # half-zeroed so each matmul picks one quarter. psum rows = [Vr ; Vi].
    ghrA = np.zeros((H, 112), np.float32)
    ghrA[:, :H] = Chi.T
    ghrA[:, H:] = Shi.T
    ghiA = np.zeros((H, 112), np.float32)
    ghiA[:, :H] = -Shi.T
    ghiA[:, H:] = Chi.T
    ghra = np.zeros((112, 112), np.float32)
    ghra[0:56] = ghrA
    ghrb = np.zeros((112, 112), np.float32)
    ghrb[56:112] = ghrA
    ghia = np.zeros((112, 112), np.float32)
    ghia[0:56] = ghiA
    ghib = np.zeros((112, 112), np.float32)
    ghib[56:112] = ghiA

    # iW stationary, block-diag: moving rows = [Vr;Vi] of a (0:58) | b (58:116)
    gwA = np.zeros((58, W), np.float32)
    gwA[:WF] = A.T
    gwA[WF:] = Bm.T
    gw2 = np.zeros((116, 112), np.float32)
    gw2[0:58, 0:56] = gwA
    gw2[58:116, 56:112] = gwA

    # dual identity: S5 transposes read xf rows 0:120 (a at 0:56, zeros at
    # 56:64, b at 64:120); idd2 maps them to out cols (a h' 0:56 | b 56:112).
    idd2 = np.zeros((128, 112), np.float32)
    for i in range(56):
        idd2[i, i] = 1.0
        idd2[64 + i, 56 + i] = 1.0

    # Mixing weights, complex-interleaved on both sides (as v1).
    def pack_mix(wl):
        wr, wi = wl[0], wl[1]  # (NB, 192, 192)
        m = np.zeros((NB, 3, 3, 128, 128), np.float32)
        for blk in range(NB):
            for kc in range(3):
                ds = slice(64 * kc, 64 * kc + 64)
                for mc in range(3):
                    ks = slice(64 * mc, 64 * mc + 64)
                    blkr = wr[blk][ds, ks]
                    blki = wi[blk][ds, ks]
                    t = m[blk, kc, mc]
                    t[0::2, 0::2] = blkr
                    t[1::2, 0::2] = -blki
                    t[0::2, 1::2] = blki
                    t[1::2, 1::2] = blkr
        return m

    m1 = pack_mix(w1)
    m2 = pack_mix(w2)

    def pack_bias(bl, scale=1.0, off=0.0):
        out = np.zeros((NB * 3, 128), np.float32)
        for blk in range(NB):
            for mc in range(3):
                ks = slice(64 * mc, 64 * mc + 64)
                out[blk * 3 + mc, 0::2] = scale * bl[0][blk][ks] + off
                out[blk * 3 + mc, 1::2] = scale * bl[1][blk][ks] + off
        return out

    b1p = pack_bias(b1)
    b2e1 = pack_bias(b2, 1.0, -LAM)   # e1 = relu(v + b2 - lam)
    b2e2 = pack_bias(b2, -1.0, -LAM)  # ACT form: relu(-v -b2 -lam) = -e2m
    b2e3 = pack_bias(b2, 1.0, LAM)    # DVE form: min(v + b2 + lam, 0) = e2m

    cb = lambda a: np.ascontiguousarray(a.astype(BF))
    cf = lambda a: np.ascontiguousarray(a.astype(np.float32))
    return {
        "fw2": cb(fw2), "fhr": cb(fhr), "fhi": cb(fhi),
        "ghra": cb(ghra), "ghrb": cb(ghrb),
        "ghia": cb(ghia), "ghib": cb(ghib),
        "gw2": cb(gw2), "idd2": cb(idd2),
        "m1": cb(m1), "m2": cb(m2),
        "b1p": cf(b1p), "b2e1": cf(b2e1), "b2e2": cf(b2e2),
        "b2e3": cf(b2e3),
    }


def build_nc(n_b=BPC):
    nc = bacc.Bacc(None, target_bir_lowering=False, debug=False)

    x_ext = nc.declare_dram_parameter("x", [n_b, H, W, C], F32, isOutput=False)
    out_ext = nc.declare_dram_parameter("out", [n_b, H, W, C], F32, isOutput=True)
    fw2_e = nc.declare_dram_parameter("fw2", [112, 116], BF16, isOutput=False)
    fhr_e = nc.declare_dram_parameter("fhr", [112, 64], BF16, isOutput=False)
    fhi_e = nc.declare_dram_parameter("fhi", [112, 64], BF16, isOutput=False)
    ghra_e = nc.declare_dram_parameter("ghra", [112, 112], BF16, isOutput=False)
    ghrb_e = nc.declare_dram_parameter("ghrb", [112, 112], BF16, isOutput=False)
    ghia_e = nc.declare_dram_parameter("ghia", [112, 112], BF16, isOutput=False)
    ghib_e = nc.declare_dram_parameter("ghib", [112, 112], BF16, isOutput=False)
    gw2_e = nc.declare_dram_parameter("gw2", [116, 112], BF16, isOutput=False)
    idd2_e = nc.declare_dram_parameter("idd2", [128, 112], BF16, isOutput=False)
    m1_e = nc.declare_dram_parameter("m1", [NB, 3, 3, 128, 128], BF16, isOutput=False)
    m2_e = nc.declare_dram_parameter("m2", [NB, 3, 3, 128, 128], BF16, isOutput=False)
    b1p_e = nc.declare_dram_parameter("b1p", [NB * 3, 128], F32, isOutput=False)
    b2e1_e = nc.declare_dram_parameter("b2e1", [NB * 3, 128], F32, isOutput=False)
    b2e2_e = nc.declare_dram_parameter("b2e2", [NB * 3, 128], F32, isOutput=False)
    b2e3_e = nc.declare_dram_parameter("b2e3", [NB * 3, 128], F32, isOutput=False)

    ev = [0]

    with tile.TileContext(nc) as tc, ExitStack() as ctx:
        consts = ctx.enter_context(tc.tile_pool(name="consts", bufs=1))
        io = ctx.enter_context(tc.tile_pool(name="io", bufs=1))
        mid = ctx.enter_context(tc.tile_pool(name="mid", bufs=1))
        mix = ctx.enter_context(tc.tile_pool(name="mix", bufs=1))
        ps = ctx.enter_context(tc.tile_pool(name="ps", bufs=1, space="PSUM"))
        dram = ctx.enter_context(tc.tile_pool(name="dram", bufs=2, space="DRAM"))

        # ---- load constants ----
        fw2_t = consts.tile([112, 116], BF16, tag="c1")
        nc.sync.dma_start(out=fw2_t, in_=fw2_e[:, :])
        fhr_t = consts.tile([112, 64], BF16, tag="c2")
        nc.sync.dma_start(out=fhr_t, in_=fhr_e[:, :])
        fhi_t = consts.tile([112, 64], BF16, tag="c3")
        nc.sync.dma_start(out=fhi_t, in_=fhi_e[:, :])
        ghra_t = consts.tile([112, 112], BF16, tag="c4")
        nc.sync.dma_start(out=ghra_t, in_=ghra_e[:, :])
        ghrb_t = consts.tile([112, 112], BF16, tag="c5")
        nc.sync.dma_start(out=ghrb_t, in_=ghrb_e[:, :])
        ghia_t = consts.tile([112, 112], BF16, tag="c6")
        nc.sync.dma_start(out=ghia_t, in_=ghia_e[:, :])
        ghib_t = consts.tile([112, 112], BF16, tag="c7")
        nc.sync.dma_start(out=ghib_t, in_=ghib_e[:, :])
        gw2_t = consts.tile([116, 112], BF16, tag="c8")
        nc.sync.dma_start(out=gw2_t, in_=gw2_e[:, :])
        idd2_t = consts.tile([128, 112], BF16, tag="c9")
        nc.sync.dma_start(out=idd2_t, in_=idd2_e[:, :])
        m1_t = consts.tile([128, NB, 3, 3, 128], BF16, tag="ca")
        nc.sync.dma_start(out=m1_t, in_=m1_e[:, :, :, :, :].transpose((3, 0, 1, 2, 4)))
        m2_t = consts.tile([128, NB, 3, 3, 128], BF16, tag="cb")
        nc.sync.dma_start(out=m2_t, in_=m2_e[:, :, :, :, :].transpose((3, 0, 1, 2, 4)))
        b1p_t = consts.tile([128, NB * 3], F32, tag="cc")
        nc.sync.dma_start(out=b1p_t, in_=b1p_e[:, :].transpose((1, 0)))
        b2e1_t = consts.tile([128, NB * 3], F32, tag="cd")
        nc.sync.dma_start(out=b2e1_t, in_=b2e1_e[:, :].transpose((1, 0)))
        b2e2_t = consts.tile([128, NB * 3], F32, tag="ce")
        nc.sync.dma_start(out=b2e2_t, in_=b2e2_e[:, :].transpose((1, 0)))
        b2e3_t = consts.tile([128, NB * 3], F32, tag="cg")
        nc.sync.dma_start(out=b2e3_t, in_=b2e3_e[:, :].transpose((1, 0)))
        ident = consts.tile([128, 128], BF16, tag="cf")
        make_identity(nc, ident[:, :])

        # greedy DVE/ACT load balancer (est. ns per engine)
        load = {"dve": 0.0, "act": 0.0, "pool": 0.0}

        def pick(n, dve_fixed=170.0, act_fixed=218.0):
            cd = load["dve"] + n * 1.04 + dve_fixed
            ca = load["act"] + n * 0.833 + act_fixed
            if cd <= ca:
                load["dve"] = cd
                return "dve"
            load["act"] = ca
            return "act"

        def evict(dst, src):
            n = src.free_size()
            if pick(n) == "dve":
                nc.vector.tensor_copy(dst, src)
            else:
                nc.scalar.activation(dst, src, AF.Copy)

        for b in range(n_b):
            for g in range(2):
                cqa, cqb = 2 * g, 2 * g + 1
                cs_a = slice(cqa * BS, cqa * BS + BS)
                cs_b = slice(cqb * BS, cqb * BS + BS)

                # ---- S1+S2+rot1: load x h-chunks, Wfft, bounce to DRAM,
                # transposing read-back per chunk
                s1 = dram.tile([116, H, BS], BF16, tag="s1")
                yh = [mid.tile([112, WF, BS], BF16, tag=f"h{Hh}",
                               name=f"yh{b}{g}{Hh}") for Hh in range(2)]
                for ch in range(NCHI):
                    hs = slice(HCHI * ch, HCHI * ch + HCHI)
                    xw = io.tile([112, HCHI, BS], BF16, tag="xw",
                                 bufs=1 if (BIG_ROT4 or BIG_LOAD) else 2,
                                 name=f"xw{b}{g}{ch}")
                    nc.gpsimd.dma_start(
                        out=xw[0:56, :, :],
                        in_=x_ext[b, hs, :, cs_a].transpose((1, 0, 2)))
                    nc.gpsimd.dma_start(
                        out=xw[56:112, :, :],
                        in_=x_ext[b, hs, :, cs_b].transpose((1, 0, 2)))
                    load["pool"] += 2520.0
                    xw_f = xw[:, :, :].rearrange("w h c -> w (h c)")
                    ywc = io.tile([116, HCHI, BS], BF16, tag="yw",
                                  bufs=1 if (BIG_ROT4 or BIG_LOAD) else 2,
                                  name=f"yw{b}{g}{ch}")
                    ywc_f = ywc[:, :, :].rearrange("p h c -> p (h c)")
                    for s in range(HCHI * BS // 448):
                        sl = slice(448 * s, 448 * (s + 1))
                        pw = ps.tile([128, 512], F32, tag="mm", bufs=2,
                                     name=f"pw{b}{g}{ch}{s}")
                        nc.tensor.matmul(pw[0:116, 0:448], fw2_t[:, :],
                                         xw_f[:, sl], start=True, stop=True)
                        evict(ywc_f[:, sl], pw[0:116, 0:448])
                    nc.sync.dma_start(out=s1[:, hs, :], in_=ywc)
                    if CHUNKED_ROT1:
                        # rot1r chunk: s1 h-slice -> YH_a/b partition slices
                        for Hh, ro in ((0, 0), (1, 58)):
                            for r in range(2):
                                nc.sync.dma_start(
                                    out=yh[Hh][56 * r + HCHI * ch:
                                               56 * r + HCHI * ch + HCHI, :, :],
                                    in_=s1[ro + 29 * r:ro + 29 * r + 29, hs, :]
                                    .transpose((1, 0, 2)))
                if not CHUNKED_ROT1:
                    for Hh, ro in ((0, 0), (1, 58)):
                        for r in range(2):
                            nc.sync.dma_start(
                                out=yh[Hh][56 * r:56 * r + 56, :, :],
                                in_=s1[ro + 29 * r:ro + 29 * r + 29, :, :]
                                .transpose((1, 0, 2)))
                yh_f = [t[:, :, :].rearrange("p w c -> p (w c)") for t in yh]

                # ---- S4: Hfft -> XF [128=(a,0,b,0) | w', c, r]
                xf = mid.tile([128, WF, BS, 2], BF16, tag="big",
                              name=f"xf{b}{g}")
                xf_r = xf[:, :, :, 0].rearrange("q w c -> q (w c)")
                xf_i = xf[:, :, :, 1].rearrange("q w c -> q (w c)")
                for j in range(11):
                    lo = 512 * j
                    hi = min(512 * (j + 1), NWC)
                    n = hi - lo
                    sl = slice(lo, hi)
                    s4tag = "mx" if (PS_ALT and j % 2) else "mm"
                    pr = ps.tile([128, 512], F32, tag=s4tag, bufs=2,
                                 name=f"pr{b}{g}{j}")
                    pi = ps.tile([128, 512], F32, tag=s4tag, bufs=2,
                                 name=f"pi{b}{g}{j}")
                    nc.tensor.matmul(pr[0:64, :n], fhr_t[:, :], yh_f[0][:, sl],
                                     start=True, stop=True)
                    nc.tensor.matmul(pr[64:128, :n], fhr_t[:, :], yh_f[1][:, sl],
                                     start=True, stop=True)
                    nc.tensor.matmul(pi[0:64, :n], fhi_t[:, :], yh_f[0][:, sl],
                                     start=True, stop=True)
                    nc.tensor.matmul(pi[64:128, :n], fhi_t[:, :], yh_f[1][:, sl],
                                     start=True, stop=True)
                    evict(xf_r[:, sl], pr[:, :n])
                    evict(xf_i[:, sl], pi[:, :n])

                # ---- S5: corner turn -> XM[kc] [128=(2c+r) | (H, w', h)]
                # one transpose covers both halves via idd2.
                xm = []
                for kc in range(3):
                    t = mix.tile([128, 2, WF, H], BF16, tag=f"xm{kc}", bufs=2,
                                 name=f"xm{b}{g}{kc}")
                    xm.append(t)
                    src = xf[0:120, :, :, :].rearrange("q w c r -> q w (c r)")
                    dst = t[:, :, :, :]
                    wp = 0
                    for grp in (4, 4, 4, 4, 4, 4, 4, 1):
                        if wp >= WF:
                            break
                        npx = min(grp, WF - wp)
                        t5tag = "t8" if (PS_ALT and (wp // 4) % 2) else "tr"
                        pt = ps.tile([128, 4, 112], F32, tag=t5tag,
                                     bufs=1 if KC_OUTER else 2,
                                     name=f"pt{b}{g}{kc}{wp}")
                        for i in range(npx):
                            nc.tensor.matmul(
                                pt[:, i, :], src[:, wp + i,
                                                 128 * kc:128 * kc + 128],
                                idd2_t[0:120, :], start=True, stop=True)
                        # evict: src (wp, H, h) -> dst (H, w', h) transposed view
                        evict(dst[:, :, wp:wp + npx, :].transpose((0, 2, 1, 3)),
                              pt[:, 0:npx, :].rearrange("p w q -> p (w q)"))
                        wp += npx
                xm_f = [t[:, :, :, :].rearrange("p H w h -> p (H w h)")
                        for t in xm]

                # ---- M1/M2 per half
                hm = [mix.tile([128, 2, WF, H], BF16, tag=f"hm{kc}", bufs=1,
                               name=f"hm{b}{g}{kc}") for kc in range(3)]
                hm_f = [t[:, :, :, :].rearrange("p H w h -> p (H w h)")
                        for t in hm]
                # om layout (w', H, h): S8's stationary slice must be a
                # single contiguous free dim (walrus 2D-weights rule).
                om = [mix.tile([128, WF, 2, H], BF16, tag=f"xm{kc}", bufs=2,
                               name=f"om{b}{g}{kc}") for kc in range(3)]
                JT = (448, 448, 448, 280)
                JW = (8, 8, 8, 5)
                for Hh in range(2):
                    cq = 2 * g + Hh
                    base = SPA * Hh
                    for mc in range(3):
                        bidx = cq * 3 + mc
                        pms = [ps.tile([128, 448], F32, tag="mx",
                                       bufs=4 if KC_OUTER else 2,
                                       name=f"pm{b}{g}{Hh}{mc}{j}")
                               for j in range(4)]
                        if KC_OUTER:
                            for kc in range(3):
                                for j in range(4):
                                    nc.tensor.matmul(
                                        pms[j][:, :JT[j]],
                                        m1_t[:, cq, kc, mc, :],
                                        xm_f[kc][:, base + 448 * j:
                                                 base + 448 * j + JT[j]],
                                        start=(kc == 0), stop=(kc == 2))
                        for j in range(4):
                            lo = base + 448 * j
                            n = JT[j]
                            sl = slice(lo, lo + n)
                            pm = pms[j]
                            if not KC_OUTER:
                                for kc in range(3):
                                    nc.tensor.matmul(
                                        pm[:, :n], m1_t[:, cq, kc, mc, :],
                                        xm_f[kc][:, sl],
                                        start=(kc == 0), stop=(kc == 2))
                            if pick(n) == "dve":
                                nc.vector.tensor_scalar(
                                    hm_f[mc][:, sl], pm[:, :n],
                                    b1p_t[:, bidx:bidx + 1], 0.0,
                                    ALU.add, ALU.max)
                            else:
                                nc.scalar.activation(
                                    hm_f[mc][:, sl], pm[:, :n], AF.Relu,
                                    bias=b1p_t[:, bidx:bidx + 1], scale=1.0)
                for Hh in range(2):
                    cq = 2 * g + Hh
                    base = SPA * Hh
                    for mc in range(3):
                        bidx = cq * 3 + mc
                        qms = [ps.tile([128, 448], F32, tag="mx",
                                       bufs=4 if KC_OUTER else 2,
                                       name=f"qm{b}{g}{Hh}{mc}{j}")
                               for j in range(4)]
                        if KC_OUTER:
                            for kc in range(3):
                                for j in range(4):
                                    nc.tensor.matmul(
                                        qms[j][:, :JT[j]],
                                        m2_t[:, cq, kc, mc, :],
                                        hm_f[kc][:, base + 448 * j:
                                                 base + 448 * j + JT[j]],
                                        start=(kc == 0), stop=(kc == 2))
                        for j in range(4):
                            lo = base + 448 * j
                            n = JT[j]
                            sl = slice(lo, lo + n)
                            pm = qms[j]
                            if not KC_OUTER:
                                for kc in range(3):
                                    nc.tensor.matmul(
                                        pm[:, :n], m2_t[:, cq, kc, mc, :],
                                        hm_f[kc][:, sl],
                                        start=(kc == 0), stop=(kc == 2))
                            e2 = mix.tile([128, 448], BF16, tag="e2", bufs=2,
                                          name=f"e2{b}{g}{Hh}{mc}{j}")
                            omd = om[mc][:, 8 * j:8 * j + JW[j], Hh, :]
                            # e1 = relu(v+b2-lam) -> om
                            if pick(n) == "dve":
                                nc.vector.tensor_scalar(
                                    omd, pm[:, :n],
                                    b2e1_t[:, bidx:bidx + 1], 0.0,
                                    ALU.add, ALU.max)
                            else:
                                nc.scalar.activation(
                                    omd, pm[:, :n], AF.Relu,
                                    bias=b2e1_t[:, bidx:bidx + 1], scale=1.0)
                            # e2m = min(v+b2+lam, 0) (DVE) or -e2m (ACT)
                            if pick(n) == "dve":
                                nc.vector.tensor_scalar(
                                    e2[:, :n], pm[:, :n],
                                    b2e3_t[:, bidx:bidx + 1], 0.0,
                                    ALU.add, ALU.min)
                                cop = ALU.add
                            else:
                                nc.scalar.activation(
                                    e2[:, :n], pm[:, :n], AF.Relu,
                                    bias=b2e2_t[:, bidx:bidx + 1], scale=-1.0)
                                cop = ALU.subtract
                            # om = e1 +/- e2 on DVE or Pool (gpsimd)
                            cd = load["dve"] + n * 1.04 + 170
                            cp = load["pool"] + n * 1.98 + 131
                            if USE_POOL_TT and cp < cd:
                                load["pool"] = cp
                                nc.gpsimd.tensor_tensor(omd, omd, e2[:, :n],
                                                        cop)
                            else:
                                load["dve"] = cd
                                nc.vector.tensor_tensor(omd, omd, e2[:, :n],
                                                        cop)

                # ---- S8: corner turn back -> OC [112=(a h'|b h') | w', c, r]
                oc = mid.tile([112, WF, BS, 2], BF16, tag="big",
                              name=f"oc{b}{g}")
                for mc in range(3):
                    wp = 0
                    for grp in (8, 8, 8, 5):
                        if wp >= WF:
                            break
                        npx = min(grp, WF - wp)
                        t8tag = "tr" if (PS_ALT and (wp // 8) % 2) else "t8"
                        pt8 = ps.tile([112, 8, 128], BF16, tag=t8tag,
                                      bufs=1 if KC_OUTER else 2,
                                      name=f"p8{b}{g}{mc}{wp}")
                        for i in range(npx):
                            # lhsT: om [128 | (H, h)] pair for this w'
                            nc.tensor.matmul(
                                pt8[:, i, :],
                                om[mc][:, wp + i, :, :]
                                .rearrange("p H h -> p (H h)"),
                                ident[:, :], is_transpose=True)
                        evict(oc[:, wp:wp + npx, 64 * mc:64 * mc + 64, :],
                              pt8[:, 0:npx, :].rearrange("p w q -> p (w q)"))
                        wp += npx
                oc_r = oc[:, :, :, 0].rearrange("q w c -> q (w c)")
                oc_i = oc[:, :, :, 1].rearrange("q w c -> q (w c)")

                # ---- S9: iH -> VH_a/b ((r,h) | w', c)
                vh = []
                for Hh, gr, gi in ((0, ghra_t, ghia_t), (1, ghrb_t, ghib_t)):
                    t = mid.tile([112, WF, BS], BF16, tag=f"h{Hh}",
                                 name=f"vh{b}{g}{Hh}")
                    t_f = t[:, :, :].rearrange("p w c -> p (w c)")
                    for j in range(11):
                        lo = 512 * j
                        hi = min(512 * (j + 1), NWC)
                        n = hi - lo
                        sl = slice(lo, hi)
                        pv = ps.tile([128, 512], F32, tag="mm", bufs=2,
                                     name=f"pv{b}{g}{Hh}{j}")
                        nc.tensor.matmul(pv[0:112, :n], gr[:, :], oc_r[:, sl],
                                         start=True, stop=False)
                        nc.tensor.matmul(pv[0:112, :n], gi[:, :], oc_i[:, sl],
                                         start=False, stop=True)
                        evict(t_f[:, sl], pv[0:112, :n])
                    vh.append(t)

                # ---- rot4: VH -> DRAM s4 -> VW chunks; S11+store per chunk
                s4 = []
                for Hh in range(2):
                    t = dram.tile([112, WF, BS], BF16, tag=f"s4{Hh}")
                    nc.sync.dma_start(out=t[:, :, :], in_=vh[Hh][:, :, :])
                    s4.append(t)
                vwf = None
                if BIG_ROT4:
                    vwf = io.tile([116, H, BS], BF16, tag="vw", bufs=1,
                                  name=f"vw{b}{g}")
                    for Hh in range(2):
                        for r in range(2):
                            nc.sync.dma_start(
                                out=vwf[58 * Hh + 29 * r:
                                        58 * Hh + 29 * r + 29, :, :],
                                in_=s4[Hh][56 * r:56 * r + 56, :, :]
                                .transpose((1, 0, 2)))
                for ch in range(NCH):
                    hs = slice(HCH * ch, HCH * ch + HCH)
                    if BIG_ROT4:
                        vwc = vwf[:, hs, :]
                    else:
                        vwc = io.tile([116, HCH, BS], BF16, tag="vw", bufs=2,
                                      name=f"vw{b}{g}{ch}")
                        for Hh in range(2):
                            for r in range(2):
                                nc.sync.dma_start(
                                    out=vwc[58 * Hh + 29 * r:
                                            58 * Hh + 29 * r + 29, :, :],
                                    in_=s4[Hh][56 * r + HCH * ch:
                                               56 * r + HCH * ch + HCH, :, :]
                                    .transpose((1, 0, 2)))
                    vwc_f = vwc[:, :, :].rearrange("p h c -> p (h c)")
                    outc = io.tile([112, HCH, BS],
                                   F32 if SYNC_STORE else BF16,
                                   tag="out", bufs=1 if SYNC_STORE else 2,
                                   name=f"out{b}{g}{ch}")
                    outc_f = outc[:, :, :].rearrange("w h c -> w (h c)")
                    for s in range(6):
                        sl = slice(448 * s, 448 * (s + 1))
                        po = ps.tile([128, 512], F32, tag="mm", bufs=2,
                                     name=f"po{b}{g}{ch}{s}")
                        nc.tensor.matmul(po[0:112, 0:448], gw2_t[:, :],
                                         vwc_f[:, sl], start=True, stop=True)
                        evict(outc_f[:, sl], po[0:112, 0:448])
                    if SYNC_STORE:
                        nc.sync.dma_start(
                            out=out_ext[b, hs, :, cs_a].transpose((1, 0, 2)),
                            in_=outc[0:56, :, :])
                        nc.sync.dma_start(
                            out=out_ext[b, hs, :, cs_b].transpose((1, 0, 2)),
                            in_=outc[56:112, :, :])
                    else:
                        nc.gpsimd.dma_start(
                            out=out_ext[b, hs, :, cs_a].transpose((1, 0, 2)),
                            in_=outc[0:56, :, :])
                        nc.gpsimd.dma_start(
                            out=out_ext[b, hs, :, cs_b].transpose((1, 0, 2)),
                            in_=outc[56:112, :, :])
                        load["pool"] += 2520.0

    nc.compile()
    return nc


_NC_CACHE = {}


def _get_nc(n_b=BPC):
    if n_b not in _NC_CACHE:
        _NC_CACHE[n_b] = build_nc(n_b)
    return _NC_CACHE[n_b]


def kernel(x, w1, b1, w2, b2):
    x = np.ascontiguousarray(np.asarray(x, dtype=np.float32))
    B, N, Cc = x.shape
    consts = make_consts(np.asarray(w1), np.asarray(b1),
                         np.asarray(w2), np.asarray(b2))
    nc = _get_nc(BPC)
    in_maps = []
    for core in range(NCORES):
        shard = np.ascontiguousarray(
            x[core * BPC:(core + 1) * BPC].reshape(BPC, H, W, Cc))
        m = {"x": shard}
        m.update(consts)
        in_maps.append(m)
    res = run_bass_kernel_spmd(nc, in_maps, core_ids=list(range(NCORES)))
    out = np.concatenate(
        [res.results[i]["out"].reshape(BPC, N, Cc) for i in range(NCORES)],
        axis=0)
    return out.astype(np.float32)



# revision 2
# speedup vs baseline: 5.2704x; 5.2704x over previous
"""AFNO (Adaptive Fourier Neural Operator) Trainium2 kernel, v3.

Data-parallel over batch: 32 batches -> 8 cores x 4 batches.
Per core: 4 batches x 2 cq-pair groups. Each group processes TWO c-quarters
(blocks) at once. See v2 docstring for the pipeline; v3 changes:

  - rot4 bounce restructured: s4 DRAM laid out (Hh, r, w', h, c) so the
    read back is ONE nearly-contiguous DMA per chunk (116 descs x 5376B)
    instead of 4 strided DMAs; the stores pay the transpose instead
    (4 strided DMAs/group, same total descriptor work as before).
  - constants packed host-side into 3 tiles / 3 DMAs (DFT pack, m1+m2
    pre-transposed contiguous, biases pack).
  - DMA issue spread across SP / Pool / ACT queues (greedy est. balance);
    PSUM->SBUF evictions balanced between DVE and ACT as in v2.
"""
import numpy as np
import ml_dtypes
from contextlib import ExitStack

import concourse.bass as bass
import concourse.mybir as mybir
import concourse.tile as tile
from concourse import bacc
from concourse.bass_utils import run_bass_kernel_spmd
from concourse.masks import make_identity

H = 56
W = 56
WF = 29
C = 768
NB = 4
BS = 192
LAM = 0.01
NCORES = 8
B_FULL = 32
BPC = B_FULL // NCORES  # 4
NCH = 4                 # h-chunks per group
HCH = H // NCH          # 14
NHC = H * BS
NWC = WF * BS           # 5568
SPA = H * WF            # 1624 spatial per half

F32 = mybir.dt.float32
BF16 = mybir.dt.bfloat16
AF = mybir.ActivationFunctionType
ALU = mybir.AluOpType

BF = ml_dtypes.bfloat16

# DFT-pack column offsets
DFT_COLS = {}
_off = 0
for _name, _w in (("fw2", 116), ("fhr", 64), ("fhi", 64), ("ghra", 112),
                  ("ghrb", 112), ("ghia", 112), ("ghib", 112), ("gw2", 112),
                  ("idd2", 112)):
    DFT_COLS[_name] = (_off, _w)
    _off += _w
DFT_W = _off  # 916


def make_consts(w1, b1, w2, b2):
    """Pack DFT matrices and mixing weights/biases host-side (numpy)."""
    w = np.arange(W)
    wp = np.arange(WF)
    ang = 2 * np.pi * np.outer(wp, w) / W
    Cw = np.cos(ang) / np.sqrt(W)
    Sw = np.sin(ang) / np.sqrt(W)
    h = np.arange(H)
    angh = 2 * np.pi * np.outer(h, h) / H
    Ch = np.cos(angh) / np.sqrt(H)
    Sh = np.sin(angh) / np.sqrt(H)
    Chi, Shi = Ch, Sh
    alpha = np.full(WF, 2.0)
    alpha[0] = 1.0
    alpha[WF - 1] = 1.0
    A = (alpha[None, :] * np.cos(2 * np.pi * np.outer(w, wp) / W)) / np.sqrt(W)
    Bm = (-alpha[None, :] * np.sin(2 * np.pi * np.outer(w, wp) / W)) / np.sqrt(W)
    Bm[:, 0] = 0.0
    Bm[:, WF - 1] = 0.0

    # W-fft stationary, block-diag for the two stacked c-quarters.
    fwA = np.zeros((W, 58), np.float32)
    fwA[:, :WF] = Cw.T
    fwA[:, WF:] = -Sw.T
    fw2 = np.zeros((112, 116), np.float32)
    fw2[0:56, 0:58] = fwA
    fw2[56:112, 58:116] = fwA

    # H-fft stationaries, moving rows = [Yr(h) ; Yi(h)] (112), 8 zero pad.
    fhr = np.zeros((112, 64), np.float32)
    fhr[:H, :H] = Ch.T
    fhr[H:, :H] = Sh.T
    fhi = np.zeros((112, 64), np.float32)
    fhi[:H, :H] = -Sh.T
    fhi[H:, :H] = Ch.T

    # iH stationaries, half-zeroed per quarter; psum rows = [Vr ; Vi].
    ghrA = np.zeros((H, 112), np.float32)
    ghrA[:, :H] = Chi.T
    ghrA[:, H:] = Shi.T
    ghiA = np.zeros((H, 112), np.float32)
    ghiA[:, :H] = -Shi.T
    ghiA[:, H:] = Chi.T
    ghra = np.zeros((112, 112), np.float32)
    ghra[0:56] = ghrA
    ghrb = np.zeros((112, 112), np.float32)
    ghrb[56:112] = ghrA
    ghia = np.zeros((112, 112), np.float32)
    ghia[0:56] = ghiA
    ghib = np.zeros((112, 112), np.float32)
    ghib[56:112] = ghiA

    # iW stationary, block-diag.
    gwA = np.zeros((58, W), np.float32)
    gwA[:WF] = A.T
    gwA[WF:] = Bm.T
    gw2 = np.zeros((116, 112), np.float32)
    gw2[0:58, 0:56] = gwA
    gw2[58:116, 56:112] = gwA

    # dual identity for S5 transposes.
    idd2 = np.zeros((128, 112), np.float32)
    for i in range(56):
        idd2[i, i] = 1.0
        idd2[64 + i, 56 + i] = 1.0

    dft = np.zeros((128, DFT_W), np.float32)
    for name, mat in (("fw2", fw2), ("fhr", fhr), ("fhi", fhi),
                      ("ghra", ghra), ("ghrb", ghrb), ("ghia", ghia),
                      ("ghib", ghib), ("gw2", gw2), ("idd2", idd2)):
        off, wd = DFT_COLS[name]
        dft[0:mat.shape[0], off:off + wd] = mat

    # Mixing weights, complex-interleaved on both sides, pre-transposed to
    # contraction-major (128, NB, 3, 3, 128) so the load is contiguous.
    def pack_mix(wl):
        wr, wi = wl[0], wl[1]  # (NB, 192, 192)
        m = np.zeros((NB, 3, 3, 128, 128), np.float32)
        for blk in range(NB):
            for kc in range(3):
                ds = slice(64 * kc, 64 * kc + 64)
                for mc in range(3):
                    ks = slice(64 * mc, 64 * mc + 64)
                    blkr = wr[blk][ds, ks]
                    blki = wi[blk][ds, ks]
                    t = m[blk, kc, mc]
                    t[0::2, 0::2] = blkr
                    t[1::2, 0::2] = -blki
                    t[0::2, 1::2] = blki
                    t[1::2, 1::2] = blkr
        return m.transpose((3, 0, 1, 2, 4))  # (128, NB, 3, 3, 128)

    m12 = np.stack([pack_mix(w1), pack_mix(w2)], axis=1)  # (128, 2, NB,3,3,128)

    def pack_bias(bl, scale=1.0, off=0.0):
        out = np.zeros((NB * 3, 128), np.float32)
        for blk in range(NB):
            for mc in range(3):
                ks = slice(64 * mc, 64 * mc + 64)
                out[blk * 3 + mc, 0::2] = scale * bl[0][blk][ks] + off
                out[blk * 3 + mc, 1::2] = scale * bl[1][blk][ks] + off
        return out.T  # (128, 12)

    bias = np.stack([
        pack_bias(b1),               # 0: b1p (relu bias)
        pack_bias(b2, 1.0, -LAM),    # 1: e1 = relu(v + b2 - lam)
        pack_bias(b2, -1.0, -LAM),   # 2: ACT form e2
        pack_bias(b2, 1.0, LAM),     # 3: DVE form e2
    ], axis=1)  # (128, 4, 12)

    cb = lambda a: np.ascontiguousarray(a.astype(BF))
    cf = lambda a: np.ascontiguousarray(a.astype(np.float32))
    return {"dft": cb(dft), "m12": cb(m12), "bias": cf(bias)}


def build_nc(n_b=BPC, dma_queues=None):
    if dma_queues is None:
        import os as _os
        dma_queues = tuple(
            _os.environ.get("KV3_DMA_QUEUES", "sp,pool").split(","))
    nc = bacc.Bacc(None, target_bir_lowering=False, debug=False)

    x_ext = nc.declare_dram_parameter("x", [n_b, H, W, C], F32, isOutput=False)
    out_ext = nc.declare_dram_parameter("out", [n_b, H, W, C], F32, isOutput=True)
    dft_e = nc.declare_dram_parameter("dft", [128, DFT_W], BF16, isOutput=False)
    m12_e = nc.declare_dram_parameter("m12", [128, 2, NB, 3, 3, 128], BF16,
                                      isOutput=False)
    bias_e = nc.declare_dram_parameter("bias", [128, 4, NB * 3], F32,
                                       isOutput=False)

    with tile.TileContext(nc) as tc, ExitStack() as ctx:
        consts = ctx.enter_context(tc.tile_pool(name="consts", bufs=1))
        io = ctx.enter_context(tc.tile_pool(name="io", bufs=1))
        mid = ctx.enter_context(tc.tile_pool(name="mid", bufs=1))
        mix = ctx.enter_context(tc.tile_pool(name="mix", bufs=1))
        ps = ctx.enter_context(tc.tile_pool(name="ps", bufs=1, space="PSUM"))
        dram = ctx.enter_context(tc.tile_pool(name="dram", bufs=2, space="DRAM"))

        # ---- load constants (3 DMAs, spread over queues) ----
        dft_t = consts.tile([128, DFT_W], BF16, tag="c1")
        nc.sync.dma_start(out=dft_t, in_=dft_e[:, :])
        m12_t = consts.tile([128, 2, NB, 3, 3, 128], BF16, tag="c2")
        nc.scalar.dma_start(out=m12_t, in_=m12_e[:, :, :, :, :, :])
        bias_t = consts.tile([128, 4, NB * 3], F32, tag="c3")
        nc.gpsimd.dma_start(out=bias_t, in_=bias_e[:, :, :])
        ident = consts.tile([128, 128], BF16, tag="c4")
        make_identity(nc, ident[:, :])

        def dftm(name):
            off, wd = DFT_COLS[name]
            rows = {"fw2": 112, "fhr": 112, "fhi": 112, "ghra": 112,
                    "ghrb": 112, "ghia": 112, "ghib": 112, "gw2": 116,
                    "idd2": 128}[name]
            return dft_t[0:rows, off:off + wd]

        fw2_t = dftm("fw2")
        fhr_t = dftm("fhr")
        fhi_t = dftm("fhi")
        ghra_t = dftm("ghra")
        ghrb_t = dftm("ghrb")
        ghia_t = dftm("ghia")
        ghib_t = dftm("ghib")
        gw2_t = dftm("gw2")
        idd2_t = dftm("idd2")

        # greedy engine load balancer (est. ns per engine queue)
        load = {"dve": 0.0, "act": 0.0, "pool": 0.0, "sp": 0.0}

        def pick(n, dve_fixed=170.0, act_fixed=218.0):
            cd = load["dve"] + n * 1.04 + dve_fixed
            ca = load["act"] + n * 0.833 + act_fixed
            if cd <= ca:
                load["dve"] = cd
                return "dve"
            load["act"] = ca
            return "act"

        def evict(dst, src):
            n = src.free_size()
            if pick(n) == "dve":
                nc.vector.tensor_copy(dst, src)
            else:
                nc.scalar.activation(dst, src, AF.Copy)

        DMA_QUEUES = tuple(dma_queues)

        def dma(out, in_, est):
            """Issue a DMA on the least-loaded capable queue."""
            best, cost = None, None
            for q in DMA_QUEUES:
                c = load[q] + est
                if cost is None or c < cost:
                    best, cost = q, c
            load[best] = cost
            eng = {"sp": nc.sync, "pool": nc.gpsimd, "act": nc.scalar}[best]
            eng.dma_start(out=out, in_=in_)

        for b in range(n_b):
            for g in range(2):
                cqa, cqb = 2 * g, 2 * g + 1
                cs_a = slice(cqa * BS, cqa * BS + BS)
                cs_b = slice(cqb * BS, cqb * BS + BS)

                # ---- S1+S2+rot1: load x h-chunks, Wfft, bounce to DRAM,
                # transposing read-back
                s1 = dram.tile([116, H, BS], BF16, tag="s1")
                yh = [mid.tile([112, WF, BS], BF16, tag=f"h{Hh}",
                               name=f"yh{b}{g}{Hh}") for Hh in range(2)]
                for ch in range(NCH):
                    hs = slice(HCH * ch, HCH * ch + HCH)
                    xw = io.tile([112, HCH, BS], BF16, tag="xw", bufs=2,
                                 name=f"xw{b}{g}{ch}")
                    # cast f32->bf16: must be gpsimd
                    nc.gpsimd.dma_start(
                        out=xw[0:56, :, :],
                        in_=x_ext[b, hs, :, cs_a].transpose((1, 0, 2)))
                    nc.gpsimd.dma_start(
                        out=xw[56:112, :, :],
                        in_=x_ext[b, hs, :, cs_b].transpose((1, 0, 2)))
                    load["pool"] += 2 * 4145.0
                    xw_f = xw[:, :, :].rearrange("w h c -> w (h c)")
                    ywc = io.tile([116, HCH, BS], BF16, tag="yw", bufs=2,
                                  name=f"yw{b}{g}{ch}")
                    ywc_f = ywc[:, :, :].rearrange("p h c -> p (h c)")
                    for sp in range(3):
                        pw = ps.tile([128, 2, 512], F32, tag="A", bufs=4,
                                     name=f"pw{b}{g}{ch}{sp}")
                        for half in range(2):
                            s = 2 * sp + half
                            sl = slice(448 * s, 448 * (s + 1))
                            nc.tensor.matmul(pw[0:116, half, 0:448], fw2_t,
                                             xw_f[:, sl], start=True,
                                             stop=True)
                        evict(ywc_f[:, 896 * sp:896 * (sp + 1)],
                              pw[0:116, :, 0:448])
                    dma(s1[:, hs, :], ywc, 2073.0)
                for Hh, ro in ((0, 0), (1, 58)):
                    for r in range(2):
                        dma(yh[Hh][56 * r:56 * r + 56, :, :],
                            s1[ro + 29 * r:ro + 29 * r + 29, :, :]
                            .transpose((1, 0, 2)), 8587.0)
                yh_f = [t[:, :, :].rearrange("p w c -> p (w c)") for t in yh]

                # ---- S4: Hfft -> XF [128=(a,0,b,0) | w', c, r]
                xf = mid.tile([128, WF, BS, 2], BF16, tag="big",
                              name=f"xf{b}{g}")
                xf_ri = xf[:, :, :, :].rearrange("q w c r -> q r (w c)")
                for j in range(11):
                    lo = 512 * j
                    hi = min(512 * (j + 1), NWC)
                    n = hi - lo
                    sl = slice(lo, hi)
                    pq = ps.tile([128, 2, 512], F32, tag="A", bufs=4,
                                 name=f"pq{b}{g}{j}")
                    nc.tensor.matmul(pq[0:64, 0, :n], fhr_t, yh_f[0][:, sl],
                                     start=True, stop=True)
                    nc.tensor.matmul(pq[64:128, 0, :n], fhr_t, yh_f[1][:, sl],
                                     start=True, stop=True)
                    nc.tensor.matmul(pq[0:64, 1, :n], fhi_t, yh_f[0][:, sl],
                                     start=True, stop=True)
                    nc.tensor.matmul(pq[64:128, 1, :n], fhi_t, yh_f[1][:, sl],
                                     start=True, stop=True)
                    evict(xf_ri[:, :, sl], pq[:, :, :n])

                # ---- S5: corner turn -> XM[kc] [128=(2c+r) | (H, w', h)]
                xm = []
                for kc in range(3):
                    t = mix.tile([128, 2, WF, H], BF16, tag=f"xm{kc}", bufs=2,
                                 name=f"xm{b}{g}{kc}")
                    xm.append(t)
                    src = xf[0:120, :, :, :].rearrange("q w c r -> q w (c r)")
                    dst = t[:, :, :, :]
                    wp = 0
                    for grp in (8, 8, 8, 5):
                        if wp >= WF:
                            break
                        npx = min(grp, WF - wp)
                        pt = ps.tile([128, 8, 128], F32, tag="A", bufs=4,
                                     name=f"pt{b}{g}{kc}{wp}")
                        for i in range(npx):
                            nc.tensor.matmul(
                                pt[:, i, 0:112], src[:, wp + i,
                                                     128 * kc:128 * kc + 128],
                                idd2_t[0:120, :], start=True, stop=True)
                        evict(dst[:, :, wp:wp + npx, :].transpose((0, 2, 1, 3)),
                              pt[:, 0:npx, 0:112])
                        wp += npx
                xm_f = [t[:, :, :, :].rearrange("p H w h -> p (H w h)")
                        for t in xm]

                # ---- M1/M2 per half
                hm = [mix.tile([128, 2, WF, H], BF16, tag=f"hm{kc}", bufs=1,
                               name=f"hm{b}{g}{kc}") for kc in range(3)]
                hm_f = [t[:, :, :, :].rearrange("p H w h -> p (H w h)")
                        for t in hm]
                om = [mix.tile([128, WF, 2, H], BF16, tag=f"xm{kc}", bufs=2,
                               name=f"om{b}{g}{kc}") for kc in range(3)]
                JT = (448, 448, 448, 280)
                JW = (8, 8, 8, 5)
                for Hh in range(2):
                    cq = 2 * g + Hh
                    base = SPA * Hh
                    for mc in range(3):
                        bidx = cq * 3 + mc
                        for jp in range(2):
                            n0 = JT[2 * jp]
                            n1 = JT[2 * jp + 1]
                            lo = base + 896 * jp
                            pm = ps.tile([128, 2, 512], F32, tag="A", bufs=4,
                                         name=f"pm{b}{g}{Hh}{mc}{jp}")
                            for half, nn in ((0, n0), (1, n1)):
                                sl = slice(lo + 448 * half,
                                           lo + 448 * half + nn)
                                for kc in range(3):
                                    nc.tensor.matmul(
                                        pm[:, half, :nn],
                                        m12_t[:, 0, cq, kc, mc, :],
                                        xm_f[kc][:, sl],
                                        start=(kc == 0), stop=(kc == 2))
                            n = n0 + n1
                            if n1 == n0:
                                esrc = pm[:, :, 0:n0]
                                edst = hm_f[mc][:, lo:lo + n]
                                if pick(n) == "dve":
                                    nc.vector.tensor_scalar(
                                        edst, esrc,
                                        bias_t[:, 0, bidx:bidx + 1], 0.0,
                                        ALU.add, ALU.max)
                                else:
                                    nc.scalar.activation(
                                        edst, esrc, AF.Relu,
                                        bias=bias_t[:, 0, bidx:bidx + 1],
                                        scale=1.0)
                            else:
                                for half, nn in ((0, n0), (1, n1)):
                                    esrc = pm[:, half, :nn]
                                    edst = hm_f[mc][:, lo + 448 * half:
                                                    lo + 448 * half + nn]
                                    if pick(nn) == "dve":
                                        nc.vector.tensor_scalar(
                                            edst, esrc,
                                            bias_t[:, 0, bidx:bidx + 1], 0.0,
                                            ALU.add, ALU.max)
                                    else:
                                        nc.scalar.activation(
                                            edst, esrc, AF.Relu,
                                            bias=bias_t[:, 0, bidx:bidx + 1],
                                            scale=1.0)
                for Hh in range(2):
                    cq = 2 * g + Hh
                    base = SPA * Hh
                    for mc in range(3):
                        bidx = cq * 3 + mc
                        for jp in range(2):
                            n0 = JT[2 * jp]
                            n1 = JT[2 * jp + 1]
                            lo = base + 896 * jp
                            nw = (n0 + n1) // H  # 16 or 13 w' columns
                            pm = ps.tile([128, 2, 512], F32, tag="A", bufs=4,
                                         name=f"qm{b}{g}{Hh}{mc}{jp}")
                            for half, nn in ((0, n0), (1, n1)):
                                sl = slice(lo + 448 * half,
                                           lo + 448 * half + nn)
                                for kc in range(3):
                                    nc.tensor.matmul(
                                        pm[:, half, :nn],
                                        m12_t[:, 1, cq, kc, mc, :],
                                        hm_f[kc][:, sl],
                                        start=(kc == 0), stop=(kc == 2))
                            # ragged pair handled as [2, 448]+[2, 280] APs is
                            # not rectangular; use per-half APs packed into
                            # one op via the om dst (w'-aligned):
                            e2 = mix.tile([128, 2, 448], BF16, tag="e2",
                                          bufs=2, name=f"e2{b}{g}{Hh}{mc}{jp}")
                            omd = om[mc][:, 16 * jp:16 * jp + nw, Hh, :]
                            if n0 == n1:
                                esrc = pm[:, :, 0:448]
                                e2v = e2[:, :, 0:448]
                                n = n0 + n1
                                # e1 = relu(v+b2-lam) -> om
                                if pick(n) == "dve":
                                    nc.vector.tensor_scalar(
                                        omd, esrc,
                                        bias_t[:, 1, bidx:bidx + 1], 0.0,
                                        ALU.add, ALU.max)
                                else:
                                    nc.scalar.activation(
                                        omd, esrc, AF.Relu,
                                        bias=bias_t[:, 1, bidx:bidx + 1],
                                        scale=1.0)
                                if pick(n) == "dve":
                                    nc.vector.tensor_scalar(
                                        e2v, esrc,
                                        bias_t[:, 3, bidx:bidx + 1], 0.0,
                                        ALU.add, ALU.min)
                                    cop = ALU.add
                                else:
                                    nc.scalar.activation(
                                        e2v, esrc, AF.Relu,
                                        bias=bias_t[:, 2, bidx:bidx + 1],
                                        scale=-1.0)
                                    cop = ALU.subtract
                                load["dve"] += n * 1.04 + 170
                                nc.vector.tensor_tensor(omd, omd, e2v, cop)
                            else:
                                for half, nn in ((0, n0), (1, n1)):
                                    esrc = pm[:, half, :nn]
                                    e2v = e2[:, half, :nn]
                                    omh = om[mc][:, 16 * jp + 8 * half:
                                                 16 * jp + 8 * half + nn // H,
                                                 Hh, :]
                                    if pick(nn) == "dve":
                                        nc.vector.tensor_scalar(
                                            omh, esrc,
                                            bias_t[:, 1, bidx:bidx + 1], 0.0,
                                            ALU.add, ALU.max)
                                    else:
                                        nc.scalar.activation(
                                            omh, esrc, AF.Relu,
                                            bias=bias_t[:, 1, bidx:bidx + 1],
                                            scale=1.0)
                                    if pick(nn) == "dve":
                                        nc.vector.tensor_scalar(
                                            e2v, esrc,
                                            bias_t[:, 3, bidx:bidx + 1], 0.0,
                                            ALU.add, ALU.min)
                                        cop = ALU.add
                                    else:
                                        nc.scalar.activation(
                                            e2v, esrc, AF.Relu,
                                            bias=bias_t[:, 2, bidx:bidx + 1],
                                            scale=-1.0)
                                        cop = ALU.subtract
                                    load["dve"] += nn * 1.04 + 170
                                    nc.vector.tensor_tensor(omh, omh, e2v,
                                                            cop)

                # ---- S8: corner turn back -> OC [112=(a h'|b h') | w', c, r]
                oc = mid.tile([112, WF, BS, 2], BF16, tag="big",
                              name=f"oc{b}{g}")
                for mc in range(3):
                    wp = 0
                    for grp in (8, 8, 8, 5):
                        if wp >= WF:
                            break
                        npx = min(grp, WF - wp)
                        pt8 = ps.tile([112, 8, 128], BF16, tag="A", bufs=4,
                                      name=f"p8{b}{g}{mc}{wp}")
                        for i in range(npx):
                            nc.tensor.matmul(
                                pt8[:, i, :],
                                om[mc][:, wp + i, :, :]
                                .rearrange("p H h -> p (H h)"),
                                ident[:, :], is_transpose=True)
                        evict(oc[:, wp:wp + npx, 64 * mc:64 * mc + 64, :],
                              pt8[:, 0:npx, :].rearrange("p w q -> p (w q)"))
                        wp += npx
                oc_r = oc[:, :, :, 0].rearrange("q w c -> q (w c)")
                oc_i = oc[:, :, :, 1].rearrange("q w c -> q (w c)")

                # ---- S9: iH -> VH_a/b ((r,h) | w', c)
                vh = []
                for Hh, gr, gi in ((0, ghra_t, ghia_t), (1, ghrb_t, ghib_t)):
                    t = mid.tile([112, WF, BS], BF16, tag=f"h{Hh}",
                                 name=f"vh{b}{g}{Hh}")
                    t_f = t[:, :, :].rearrange("p w c -> p (w c)")
                    for jp in range(6):
                        lo = 1024 * jp
                        hi = min(1024 * (jp + 1), NWC)
                        pv = ps.tile([128, 2, 512], F32, tag="A", bufs=4,
                                     name=f"pv{b}{g}{Hh}{jp}")
                        for half in range(2):
                            l2 = lo + 512 * half
                            h2 = min(l2 + 512, NWC)
                            if l2 >= h2:
                                continue
                            sl = slice(l2, h2)
                            nc.tensor.matmul(pv[0:112, half, :h2 - l2], gr,
                                             oc_r[:, sl],
                                             start=True, stop=False)
                            nc.tensor.matmul(pv[0:112, half, :h2 - l2], gi,
                                             oc_i[:, sl],
                                             start=False, stop=True)
                        if hi - lo == 1024:
                            evict(t_f[:, lo:hi], pv[0:112, :, :])
                        else:
                            evict(t_f[:, lo:hi],
                                  pv[0:112, :, :].rearrange("p a b -> p (a b)")
                                  [:, 0:hi - lo])
                    vh.append(t)

                # ---- rot4: VH -> DRAM s4y (Hh, r, w', h, c); store pays the
                # transpose so the per-chunk read back is nearly contiguous.
                s4y = dram.tile([2, 2, WF, H, BS], BF16, tag="s4")
                for Hh in range(2):
                    for r in range(2):
                        dma(s4y[Hh, r, :, :, :].transpose((1, 0, 2)),
                            vh[Hh][56 * r:56 * r + 56, :, :], 8587.0)
                for ch in range(NCH):
                    hs = slice(HCH * ch, HCH * ch + HCH)
                    vwc = io.tile([116, HCH, BS], BF16, tag="vw", bufs=2,
                                  name=f"vw{b}{g}{ch}")
                    dma(vwc[:, :, :], s4y[:, :, :, hs, :], 2073.0)
                    vwc_f = vwc[:, :, :].rearrange("p h c -> p (h c)")
                    outc = io.tile([112, HCH, BS], F32, tag="out", bufs=2,
                                   name=f"out{b}{g}{ch}")
                    outc_f = outc[:, :, :].rearrange("w h c -> w (h c)")
                    for sp in range(3):
                        po = ps.tile([128, 2, 512], F32, tag="A", bufs=4,
                                     name=f"po{b}{g}{ch}{sp}")
                        for half in range(2):
                            s = 2 * sp + half
                            sl = slice(448 * s, 448 * (s + 1))
                            nc.tensor.matmul(po[0:112, half, 0:448], gw2_t,
                                             vwc_f[:, sl], start=True,
                                             stop=True)
                        evict(outc_f[:, 896 * sp:896 * (sp + 1)],
                              po[0:112, :, 0:448])
                    dma(out_ext[b, hs, :, cs_a].transpose((1, 0, 2)),
                        outc[0:56, :, :], 4145.0)
                    dma(out_ext[b, hs, :, cs_b].transpose((1, 0, 2)),
                        outc[56:112, :, :], 4145.0)

    nc.compile()
    return nc


_NC_CACHE = {}


def _get_nc(n_b=BPC, dma_queues=None):
    key = (n_b, dma_queues)
    if key not in _NC_CACHE:
        _NC_CACHE[key] = build_nc(n_b, dma_queues)
    return _NC_CACHE[key]


def kernel(x, w1, b1, w2, b2):
    x = np.ascontiguousarray(np.asarray(x, dtype=np.float32))
    B, N, Cc = x.shape
    consts = make_consts(np.asarray(w1), np.asarray(b1),
                         np.asarray(w2), np.asarray(b2))
    nc = _get_nc(BPC)
    in_maps = []
    for core in range(NCORES):
        shard = np.ascontiguousarray(
            x[core * BPC:(core + 1) * BPC].reshape(BPC, H, W, Cc))
        m = {"x": shard}
        m.update(consts)
        in_maps.append(m)
    res = run_bass_kernel_spmd(nc, in_maps, core_ids=list(range(NCORES)))
    out = np.concatenate(
        [res.results[i]["out"].reshape(BPC, N, Cc) for i in range(NCORES)],
        axis=0)
    return out.astype(np.float32)
